# revision 15
# baseline (speedup 1.0000x reference)
"""Bahdanau additive attention on 8 Trainium2 NeuronCores.

reference:
    q_proj = (query @ Wa_w.T + Wa_b)[:, None, :]          # [B, 1, H]
    k_proj = einsum('bsh,dh->bsd', keys, Ua_w) + Ua_b     # [B, S, H]
    scores = einsum('bsh,h->bs', tanh(q_proj + k_proj), Va_w[0]) + Va_b[0]
    attn   = softmax(scores, axis=1)                      # [B, S]
    context= einsum('bs,bsh->bh', attn, keys)             # [B, H]

Sharding: pure data parallel over B (64 -> 8 per core); weights replicated.

Device pipeline (per core, B_loc=8, S=2048, H=D=512), all matmuls bf16:
  - keysT bf16 (h on partitions) drives k_projT = Ua_w @ k^T in PSUM;
    tanh + per-partition q_proj bias fused on ScalarE.
  - scores row via M=1 matmuls (lhsT = Va column) accumulated in PSUM;
    emitted after the whole chunk's projection matmuls so the PE FIFO never
    head-of-line blocks on ScalarE.
  - p = exp(scores + Va_b) on ScalarE with fused running sum (accum_out).
    No max-subtraction: |scores| <= sum|Va| + |Va_b| ~ 23 fits fp32 exp.
  - p row transposed to columns via K=1 matmuls; context = p^T-weighted sum
    of natural-layout keys via M=1 matmuls; 1/l folded in on VectorE.
  - p-transposes / softmax tail / context are software-pipelined one chunk
    behind the projection stream so their ACT/DVE dependencies are ready by
    the time the PE reaches them.
"""

import numpy as np
import ml_dtypes

B, S, H = 64, 2048, 512
NCORES = 8
BL = B // NCORES          # batches per core
HC = H // 128             # h chunks (contraction)
DT = H // 128             # d tiles (partition dim of k_projT)
SCW = 512                 # s-chunk width (PSUM bank limit)
SC = S // SCW             # s chunks
ST = S // 128             # s tiles (for p^T / context)

_cache = {}


def _build(do_compile=True):
    import concourse.bass as bass
    import concourse.tile as tile
    from concourse import bacc, mybir

    f32 = mybir.dt.float32
    bf16 = mybir.dt.bfloat16
    AF = mybir.ActivationFunctionType
    ALU = mybir.AluOpType
    AX = mybir.AxisListType

    nc = bacc.Bacc("TRN2", target_bir_lowering=False, debug=False)

    keysT = nc.declare_dram_parameter("keysT", [BL, H, S], bf16, isOutput=False)
    keysN = nc.declare_dram_parameter("keysN", [BL, S, H], bf16, isOutput=False)
    queryT = nc.declare_dram_parameter("queryT", [H, BL], f32, isOutput=False)
    WaT = nc.declare_dram_parameter("WaT", [H, H], f32, isOutput=False)
    UaT = nc.declare_dram_parameter("UaT", [H, H], bf16, isOutput=False)
    bias_wu = nc.declare_dram_parameter("bias_wu", [128, DT], f32, isOutput=False)
    va = nc.declare_dram_parameter("va", [128, DT], bf16, isOutput=False)
    vab = nc.declare_dram_parameter("vab", [1, 1], f32, isOutput=False)
    out_ctx = nc.declare_dram_parameter("out_ctx", [BL, H], f32, isOutput=True)
    out_attn = nc.declare_dram_parameter("out_attn", [BL, S], f32, isOutput=True)

    with tile.TileContext(nc) as tc:
        with (
            tc.tile_pool(name="persist", bufs=1) as pp,
            tc.tile_pool(name="keys", bufs=2) as kp,
            tc.tile_pool(name="work", bufs=8) as wp,
            tc.tile_pool(name="accs", bufs=4) as ap,
            tc.tile_pool(name="rows", bufs=3) as rp,
            tc.tile_pool(name="outs", bufs=2) as op,
            tc.tile_pool(name="ps_main", bufs=4, space="PSUM") as ps_main,
            tc.tile_pool(name="ps_sc", bufs=1, space="PSUM") as ps_sc,
            tc.tile_pool(name="ps_tp", bufs=2, space="PSUM") as ps_tp,
            tc.tile_pool(name="ps_misc", bufs=1, space="PSUM") as ps_misc,
        ):
            # ---- constants / weights (UaT first: main matmuls need it) ----
            WaT_sb, UaT_sb, qT_sb = [], [], []
            for hc in range(HC):
                u = pp.tile([128, H], bf16, tag=f"UaT{hc}", name=f"UaT{hc}")
                nc.sync.dma_start(u[:], UaT[128 * hc : 128 * (hc + 1), :])
                UaT_sb.append(u)
            for hc in range(HC):
                w = pp.tile([128, H], f32, tag=f"WaT{hc}", name=f"WaT{hc}")
                nc.gpsimd.dma_start(w[:], WaT[128 * hc : 128 * (hc + 1), :])
                WaT_sb.append(w)
                q = pp.tile([128, BL], f32, tag=f"qT{hc}", name=f"qT{hc}")
                nc.gpsimd.dma_start(q[:], queryT[128 * hc : 128 * (hc + 1), :])
                qT_sb.append(q)
            bwu_sb = pp.tile([128, DT], f32, tag="bwu")
            nc.gpsimd.dma_start(bwu_sb[:], bias_wu[:])
            va_sb = pp.tile([128, DT], bf16, tag="va")
            nc.gpsimd.dma_start(va_sb[:], va[:])
            vab_sb = pp.tile([1, 1], f32, tag="vab")
            nc.gpsimd.dma_start(vab_sb[:], vab[:])
            ones_bf1 = pp.tile([1, 1], bf16, tag="ones_bf1")
            nc.vector.memset(ones_bf1[:], 1.0)

            # ---- q_proj^T [d, b] + (Wa_b + Ua_b) bias, per d-tile ----
            qb_sb = []
            for dt in range(DT):
                pq = ps_misc.tile([128, BL], f32, tag="pmisc", name="pq")
                dsl = slice(128 * dt, 128 * (dt + 1))
                for hc in range(HC):
                    nc.tensor.matmul(
                        pq[:], WaT_sb[hc][:, dsl], qT_sb[hc][:],
                        start=(hc == 0), stop=(hc == HC - 1),
                    )
                qb = pp.tile([128, BL], f32, tag=f"qb{dt}", name=f"qb{dt}")
                nc.scalar.activation(
                    qb[:], pq[:], AF.Identity, bias=bwu_sb[:, dt : dt + 1]
                )
                qb_sb.append(qb)

            l_parts = pp.tile([1, BL * SC], f32, tag="l_parts")

            # per-batch live state
            kT = [None] * BL      # [128, HC, S] keysT tile
            kN = [None] * BL      # [128, ST, H] natural keys tile
            p_row = [None] * BL
            pT = [None] * BL
            psc_t = {}

            def load_batch(b):
                kT[b] = []
                for ht in range(HC):
                    t = kp.tile([128, S], bf16, tag=f"kT{ht}", name=f"kT{ht}_{b}")
                    nc.sync.dma_start(
                        t[:], keysT[b, 128 * ht : 128 * (ht + 1), :]
                    )
                    kT[b].append(t)
                p_row[b] = rp.tile([1, S], bf16, tag="p_row", name=f"p_row{b}")
                pT[b] = rp.tile([128, ST], bf16, tag="pT", name=f"pT{b}")

            def load_batch_kn(b):
                kN[b] = kp.tile([128, ST, H], bf16, tag="kN", name=f"kN{b}")
                nc.sync.dma_start(
                    kN[b][:], keysN[b].rearrange("(j p) h -> p j h", p=128)
                )

            def proj_chunk(b, sc):
                """16 projection MMs + 4 tanh + 4 score MMs + exp for chunk."""
                ssl = slice(SCW * sc, SCW * (sc + 1))
                t_tiles = []
                for dt in range(DT):
                    dsl = slice(128 * dt, 128 * (dt + 1))
                    pk = ps_main.tile([128, SCW], f32, tag="pk", name="pk")
                    for hc in range(HC):
                        nc.tensor.matmul(
                            pk[:], UaT_sb[hc][:, dsl], kT[b][hc][:, ssl],
                            start=(hc == 0), stop=(hc == HC - 1),
                        )
                    t_sb = wp.tile([128, SCW], bf16, tag="t", name="t")
                    nc.scalar.activation(
                        t_sb[:], pk[:], AF.Tanh, bias=qb_sb[dt][:, b : b + 1]
                    )
                    t_tiles.append(t_sb)
                psc = ps_sc.tile([1, SCW], f32, tag="psc", name=f"psc{b}_{sc}")
                for dt in range(DT):
                    nc.tensor.matmul(
                        psc[:], va_sb[:, dt : dt + 1], t_tiles[dt][:],
                        start=(dt == 0), stop=(dt == DT - 1),
                    )
                col = SC * b + sc
                nc.scalar.activation(
                    p_row[b][0:1, ssl], psc[:], AF.Exp, bias=vab_sb[0:1, 0:1],
                    accum_out=l_parts[0:1, col : col + 1],
                )

            def transpose_chunk(b, sc):
                """4 K=1 transpose MMs turning p chunk into pT columns."""
                for c in range(SCW // 128):
                    j = (SCW // 128) * sc + c
                    ptp = ps_tp.tile([128, 1], f32, tag="ptp", name="ptp")
                    nc.tensor.matmul(
                        ptp[:],
                        p_row[b][0:1, 128 * j : 128 * (j + 1)],
                        ones_bf1[:],
                        start=True, stop=True,
                    )
                    nc.vector.tensor_copy(pT[b][:, j : j + 1], ptp[:])

            def finish_batch(b):
                """softmax normalization + outputs for batch b."""
                lsum = rp.tile([1, 1], f32, tag="lsum", name=f"lsum{b}")
                nc.vector.tensor_reduce(
                    lsum[:], l_parts[0:1, SC * b : SC * (b + 1)], AX.X, ALU.add
                )
                linv_b = rp.tile([1, 1], f32, tag="linv", name=f"linv{b}")
                nc.vector.reciprocal(linv_b[:], lsum[:])
                attn_row = op.tile([1, S], f32, tag="attn_row", name="attn_row")
                nc.vector.tensor_scalar_mul(attn_row[:], p_row[b][:], linv_b[:])
                nc.sync.dma_start(out_attn[b : b + 1, :], attn_row[:])

                pctx = ps_misc.tile([1, H], f32, tag="pmisc", name=f"pctx{b}")
                for j in range(ST):
                    nc.tensor.matmul(
                        pctx[:], pT[b][:, j : j + 1], kN[b][:, j, :],
                        start=(j == 0), stop=(j == ST - 1),
                    )
                ctx_row = op.tile([1, H], f32, tag="ctx_row", name="ctx_row")
                nc.vector.tensor_scalar_mul(ctx_row[:], pctx[:], linv_b[:])
                nc.sync.dma_start(out_ctx[b : b + 1, :], ctx_row[:])

            # ---- software-pipelined emission over (batch, chunk) ----
            load_batch(0)
            load_batch_kn(0)
            chunks = [(b, sc) for b in range(BL) for sc in range(SC)]
            for g, (b, sc) in enumerate(chunks):
                if sc == 0 and b + 1 < BL:
                    load_batch(b + 1)   # prefetch next batch's keys early
                if sc == 1 and b + 1 < BL:
                    load_batch_kn(b + 1)
                proj_chunk(b, sc)
                if g >= 1:
                    pb, psc_prev = chunks[g - 1]
                    transpose_chunk(pb, psc_prev)
                    if psc_prev == SC - 1:
                        finish_batch(pb)
            transpose_chunk(BL - 1, SC - 1)
            finish_batch(BL - 1)

    if do_compile:
        nc.compile()
    return nc


def _prep_in_maps(query, keys, Wa_w, Wa_b, Ua_w, Ua_b, Va_w, Va_b):
    bf16 = ml_dtypes.bfloat16
    keysN = keys.astype(bf16)                                   # [B, S, H]
    keysT = np.ascontiguousarray(keysN.transpose(0, 2, 1))      # [B, H, S]
    queryT = np.ascontiguousarray(query.T.astype(np.float32))   # [H, B]
    WaT = np.ascontiguousarray(Wa_w.T.astype(np.float32))
    UaT = np.ascontiguousarray(Ua_w.T.astype(bf16))
    bias_wu = np.ascontiguousarray(
        (Wa_b + Ua_b).astype(np.float32).reshape(DT, 128).T
    )
    va_f = np.ascontiguousarray(Va_w[0].astype(bf16).reshape(DT, 128).T)
    vab = Va_b.reshape(1, 1).astype(np.float32)

    in_maps = []
    for c in range(NCORES):
        bs = slice(BL * c, BL * (c + 1))
        in_maps.append(
            {
                "keysT": np.ascontiguousarray(keysT[bs]),
                "keysN": np.ascontiguousarray(keysN[bs]),
                "queryT": np.ascontiguousarray(queryT[:, bs]),
                "WaT": WaT,
                "UaT": UaT,
                "bias_wu": bias_wu,
                "va": va_f,
                "vab": vab,
            }
        )
    return in_maps


def kernel(query, keys, Wa_w, Wa_b, Ua_w, Ua_b, Va_w, Va_b):
    from concourse.bass_utils import run_bass_kernel_spmd

    if "nc" not in _cache:
        _cache["nc"] = _build()
    nc = _cache["nc"]

    in_maps = _prep_in_maps(query, keys, Wa_w, Wa_b, Ua_w, Ua_b, Va_w, Va_b)
    res = run_bass_kernel_spmd(nc, in_maps, core_ids=list(range(NCORES)))
    context = np.concatenate(
        [res.results[c]["out_ctx"] for c in range(NCORES)], axis=0
    )
    attn = np.concatenate(
        [res.results[c]["out_attn"] for c in range(NCORES)], axis=0
    )
    return context, attn


# revision 16
# speedup vs baseline: 1.0334x; 1.0334x over previous
"""Bahdanau additive attention on 8 Trainium2 NeuronCores.

reference:
    q_proj = (query @ Wa_w.T + Wa_b)[:, None, :]          # [B, 1, H]
    k_proj = einsum('bsh,dh->bsd', keys, Ua_w) + Ua_b     # [B, S, H]
    scores = einsum('bsh,h->bs', tanh(q_proj + k_proj), Va_w[0]) + Va_b[0]
    attn   = softmax(scores, axis=1)                      # [B, S]
    context= einsum('bs,bsh->bh', attn, keys)             # [B, H]

Sharding: pure data parallel over B (64 -> 8 per core); weights replicated.

Device pipeline (per core, B_loc=8, S=2048, H=D=512), all matmuls bf16:
  - keysT bf16 (h on partitions) drives k_projT = Ua_w @ k^T in PSUM;
    tanh + per-partition q_proj bias fused on ScalarE.
  - scores row via M=1 matmuls (lhsT = Va column) accumulated in PSUM;
    emitted after the whole chunk's projection matmuls so the PE FIFO never
    head-of-line blocks on ScalarE.
  - p = exp(scores + Va_b) on ScalarE with fused running sum (accum_out).
    No max-subtraction: |scores| <= sum|Va| + |Va_b| ~ 23 fits fp32 exp.
  - p row transposed to columns via K=1 matmuls; context = p^T-weighted sum
    of natural-layout keys via M=1 matmuls; 1/l folded in on VectorE.
  - p-transposes / softmax tail / context are software-pipelined one chunk
    behind the projection stream so their ACT/DVE dependencies are ready by
    the time the PE reaches them.
"""

import numpy as np
import ml_dtypes

B, S, H = 64, 2048, 512
NCORES = 8
BL = B // NCORES          # batches per core
HC = H // 128             # h chunks (contraction)
DT = H // 128             # d tiles (partition dim of k_projT)
SCW = 512                 # s-chunk width (PSUM bank limit)
SC = S // SCW             # s chunks
ST = S // 128             # s tiles (for p^T / context)

_cache = {}


def _build(do_compile=True):
    import concourse.bass as bass
    import concourse.tile as tile
    from concourse import bacc, mybir

    f32 = mybir.dt.float32
    bf16 = mybir.dt.bfloat16
    AF = mybir.ActivationFunctionType
    ALU = mybir.AluOpType
    AX = mybir.AxisListType

    nc = bacc.Bacc("TRN2", target_bir_lowering=False, debug=False)

    keysT = nc.declare_dram_parameter("keysT", [BL, H, S], bf16, isOutput=False)
    keysN = nc.declare_dram_parameter("keysN", [BL, S, H], bf16, isOutput=False)
    queryT = nc.declare_dram_parameter("queryT", [H, BL], bf16, isOutput=False)
    WaT = nc.declare_dram_parameter("WaT", [H, H], bf16, isOutput=False)
    UaT = nc.declare_dram_parameter("UaT", [H, H], bf16, isOutput=False)
    bias_wu = nc.declare_dram_parameter("bias_wu", [128, DT], f32, isOutput=False)
    va = nc.declare_dram_parameter("va", [128, DT], bf16, isOutput=False)
    vab = nc.declare_dram_parameter("vab", [1, 1], f32, isOutput=False)
    out_ctx = nc.declare_dram_parameter("out_ctx", [BL, H], f32, isOutput=True)
    out_attn = nc.declare_dram_parameter("out_attn", [BL, S], f32, isOutput=True)

    with tile.TileContext(nc) as tc:
        with (
            tc.tile_pool(name="persist", bufs=1) as pp,
            tc.tile_pool(name="keys", bufs=2) as kp,
            tc.tile_pool(name="work", bufs=8) as wp,
            tc.tile_pool(name="accs", bufs=4) as ap,
            tc.tile_pool(name="rows", bufs=3) as rp,
            tc.tile_pool(name="outs", bufs=2) as op,
            tc.tile_pool(name="ps_main", bufs=3, space="PSUM") as ps_main,
            tc.tile_pool(name="ps_sc", bufs=2, space="PSUM") as ps_sc,
            tc.tile_pool(name="ps_tp", bufs=2, space="PSUM") as ps_tp,
            tc.tile_pool(name="ps_misc", bufs=1, space="PSUM") as ps_misc,
        ):
            # ---- constants / weights (q-proj weights first: they gate the
            # q matmuls that sit at the head of the PE FIFO; then UaT) ----
            WaT_sb, UaT_sb, qT_sb = [], [], []
            for hc in range(HC):
                w = pp.tile([128, H], bf16, tag=f"WaT{hc}", name=f"WaT{hc}")
                nc.sync.dma_start(w[:], WaT[128 * hc : 128 * (hc + 1), :])
                WaT_sb.append(w)
                q = pp.tile([128, BL], bf16, tag=f"qT{hc}", name=f"qT{hc}")
                nc.sync.dma_start(q[:], queryT[128 * hc : 128 * (hc + 1), :])
                qT_sb.append(q)
            for hc in range(HC):
                u = pp.tile([128, H], bf16, tag=f"UaT{hc}", name=f"UaT{hc}")
                nc.sync.dma_start(u[:], UaT[128 * hc : 128 * (hc + 1), :])
                UaT_sb.append(u)
            bwu_sb = pp.tile([128, DT], f32, tag="bwu")
            nc.gpsimd.dma_start(bwu_sb[:], bias_wu[:])
            va_sb = pp.tile([128, DT], bf16, tag="va")
            nc.gpsimd.dma_start(va_sb[:], va[:])
            vab_sb = pp.tile([1, 1], f32, tag="vab")
            nc.gpsimd.dma_start(vab_sb[:], vab[:])
            ones_bf1 = pp.tile([1, 1], bf16, tag="ones_bf1")
            nc.vector.memset(ones_bf1[:], 1.0)

            # ---- q_proj^T [d, b] + (Wa_b + Ua_b) bias, per d-tile ----
            qb_sb = []
            for dt in range(DT):
                pq = ps_misc.tile([128, BL], f32, tag="pmisc", name="pq")
                dsl = slice(128 * dt, 128 * (dt + 1))
                for hc in range(HC):
                    nc.tensor.matmul(
                        pq[:], WaT_sb[hc][:, dsl], qT_sb[hc][:],
                        start=(hc == 0), stop=(hc == HC - 1),
                    )
                qb = pp.tile([128, BL], f32, tag=f"qb{dt}", name=f"qb{dt}")
                nc.scalar.activation(
                    qb[:], pq[:], AF.Identity, bias=bwu_sb[:, dt : dt + 1]
                )
                qb_sb.append(qb)

            l_parts = pp.tile([1, BL * SC], f32, tag="l_parts")

            # per-batch live state
            kT = [None] * BL      # [128, HC, S] keysT tile
            kN = [None] * BL      # [128, ST, H] natural keys tile
            p_row = [None] * BL
            pT = [None] * BL
            psc_t = {}

            def load_batch(b):
                kT[b] = []
                for ht in range(HC):
                    t = kp.tile([128, S], bf16, tag=f"kT{ht}", name=f"kT{ht}_{b}")
                    nc.sync.dma_start(
                        t[:], keysT[b, 128 * ht : 128 * (ht + 1), :]
                    )
                    kT[b].append(t)
                p_row[b] = rp.tile([1, S], bf16, tag="p_row", name=f"p_row{b}")
                pT[b] = rp.tile([128, ST], bf16, tag="pT", name=f"pT{b}")

            def load_batch_kn(b):
                kN[b] = kp.tile([128, ST, H], bf16, tag="kN", name=f"kN{b}")
                nc.sync.dma_start(
                    kN[b][:], keysN[b].rearrange("(j p) h -> p j h", p=128)
                )

            def proj_chunk(b, sc):
                """16 projection MMs + 4 tanh + 4 score MMs + exp for chunk."""
                ssl = slice(SCW * sc, SCW * (sc + 1))
                t_tiles = []
                for dt in range(DT):
                    dsl = slice(128 * dt, 128 * (dt + 1))
                    pk = ps_main.tile([128, SCW], f32, tag="pk", name="pk")
                    for hc in range(HC):
                        nc.tensor.matmul(
                            pk[:], UaT_sb[hc][:, dsl], kT[b][hc][:, ssl],
                            start=(hc == 0), stop=(hc == HC - 1),
                        )
                    t_sb = wp.tile([128, SCW], bf16, tag="t", name="t")
                    nc.scalar.activation(
                        t_sb[:], pk[:], AF.Tanh, bias=qb_sb[dt][:, b : b + 1]
                    )
                    t_tiles.append(t_sb)
                psc = ps_sc.tile([1, SCW], f32, tag="psc", name=f"psc{b}_{sc}")
                for dt in range(DT):
                    nc.tensor.matmul(
                        psc[:], va_sb[:, dt : dt + 1], t_tiles[dt][:],
                        start=(dt == 0), stop=(dt == DT - 1),
                    )
                col = SC * b + sc
                nc.scalar.activation(
                    p_row[b][0:1, ssl], psc[:], AF.Exp, bias=vab_sb[0:1, 0:1],
                    accum_out=l_parts[0:1, col : col + 1],
                )

            def transpose_chunk(b, sc):
                """4 K=1 transpose MMs turning p chunk into pT columns."""
                for c in range(SCW // 128):
                    j = (SCW // 128) * sc + c
                    ptp = ps_tp.tile([128, 1], f32, tag="ptp", name="ptp")
                    nc.tensor.matmul(
                        ptp[:],
                        p_row[b][0:1, 128 * j : 128 * (j + 1)],
                        ones_bf1[:],
                        start=True, stop=True,
                    )
                    nc.vector.tensor_copy(pT[b][:, j : j + 1], ptp[:])

            def finish_batch(b):
                """softmax normalization + outputs for batch b."""
                lsum = rp.tile([1, 1], f32, tag="lsum", name=f"lsum{b}")
                nc.vector.tensor_reduce(
                    lsum[:], l_parts[0:1, SC * b : SC * (b + 1)], AX.X, ALU.add
                )
                linv_b = rp.tile([1, 1], f32, tag="linv", name=f"linv{b}")
                nc.vector.reciprocal(linv_b[:], lsum[:])
                attn_row = op.tile([1, S], f32, tag="attn_row", name="attn_row")
                nc.vector.tensor_scalar_mul(attn_row[:], p_row[b][:], linv_b[:])
                nc.sync.dma_start(out_attn[b : b + 1, :], attn_row[:])

                pctx = ps_misc.tile([1, H], f32, tag="pmisc", name=f"pctx{b}")
                for j in range(ST):
                    nc.tensor.matmul(
                        pctx[:], pT[b][:, j : j + 1], kN[b][:, j, :],
                        start=(j == 0), stop=(j == ST - 1),
                    )
                ctx_row = op.tile([1, H], f32, tag="ctx_row", name="ctx_row")
                nc.vector.tensor_scalar_mul(ctx_row[:], pctx[:], linv_b[:])
                nc.sync.dma_start(out_ctx[b : b + 1, :], ctx_row[:])

            # ---- software-pipelined emission over (batch, chunk) ----
            load_batch(0)
            load_batch_kn(0)
            chunks = [(b, sc) for b in range(BL) for sc in range(SC)]
            for g, (b, sc) in enumerate(chunks):
                if sc == 0 and b + 1 < BL:
                    load_batch(b + 1)   # prefetch next batch's keys early
                if sc == 1 and b + 1 < BL:
                    load_batch_kn(b + 1)
                proj_chunk(b, sc)
                if g >= 1:
                    pb, psc_prev = chunks[g - 1]
                    transpose_chunk(pb, psc_prev)
                    if psc_prev == SC - 1:
                        finish_batch(pb)
            transpose_chunk(BL - 1, SC - 1)
            finish_batch(BL - 1)

    if do_compile:
        nc.compile()
    return nc


def _prep_in_maps(query, keys, Wa_w, Wa_b, Ua_w, Ua_b, Va_w, Va_b):
    bf16 = ml_dtypes.bfloat16
    keysN = keys.astype(bf16)                                   # [B, S, H]
    keysT = np.ascontiguousarray(keysN.transpose(0, 2, 1))      # [B, H, S]
    queryT = np.ascontiguousarray(query.T.astype(bf16))         # [H, B]
    WaT = np.ascontiguousarray(Wa_w.T.astype(bf16))
    UaT = np.ascontiguousarray(Ua_w.T.astype(bf16))
    bias_wu = np.ascontiguousarray(
        (Wa_b + Ua_b).astype(np.float32).reshape(DT, 128).T
    )
    va_f = np.ascontiguousarray(Va_w[0].astype(bf16).reshape(DT, 128).T)
    vab = Va_b.reshape(1, 1).astype(np.float32)

    in_maps = []
    for c in range(NCORES):
        bs = slice(BL * c, BL * (c + 1))
        in_maps.append(
            {
                "keysT": np.ascontiguousarray(keysT[bs]),
                "keysN": np.ascontiguousarray(keysN[bs]),
                "queryT": np.ascontiguousarray(queryT[:, bs]),
                "WaT": WaT,
                "UaT": UaT,
                "bias_wu": bias_wu,
                "va": va_f,
                "vab": vab,
            }
        )
    return in_maps


def kernel(query, keys, Wa_w, Wa_b, Ua_w, Ua_b, Va_w, Va_b):
    from concourse.bass_utils import run_bass_kernel_spmd

    if "nc" not in _cache:
        _cache["nc"] = _build()
    nc = _cache["nc"]

    in_maps = _prep_in_maps(query, keys, Wa_w, Wa_b, Ua_w, Ua_b, Va_w, Va_b)
    res = run_bass_kernel_spmd(nc, in_maps, core_ids=list(range(NCORES)))
    context = np.concatenate(
        [res.results[c]["out_ctx"] for c in range(NCORES)], axis=0
    )
    attn = np.concatenate(
        [res.results[c]["out_attn"] for c in range(NCORES)], axis=0
    )
    return context, attn
